# revision 1
# baseline (speedup 1.0000x reference)
"""AttentiveStatisticsPooling Trainium2 kernel (8 NeuronCores, batch-sharded).

Reference computation (B=32, C=1536, T=2000):
    a    = einsum('bct,c->bt', x, w) + cb          # 1x1 conv -> [B,T]
    a    = BN(a)  (batch stats over all B*T, biased var)    # syncBN via AllReduce
    attn = softmax(tanh(a), axis=T)
    mean = einsum('bct,bt->bc', x, attn)
    std  = sqrt(clip(E_attn[x^2] - mean^2, 1e-10))
    out  = concat([mean, std], axis=1)             # [B, 2C]

V3 design: SINGLE HBM read of x (no second pass over HBM); a bf16 transposed
copy of the whole shard lives in SBUF.

PSUM rule honored everywhere: a matmul accumulation group owns its whole
2048-byte bank -- exactly one start (which marks the bank pending-zero) and
one stop per bank; every other matmul into that bank accumulates.

Pass 1 (per b, kc, t-chunk): DMA x tile [128c, ~1024t] f32r.
  - conv logits: matmul(lhsT=x-block [128c, tw], rhs=w-chunk [128c, 2]) f32r
    -> aT [t, 2] in transposed form; single accumulation group per sample in
    one PSUM bank (start only at (kc=0, tt=0), stop at (kc=11, tt=15)).
  - PE-transpose each [128c, 128t] block; DVE/ACT copy PSUM->SBUF converting
    to bf16 into the persistent store xts [128t, b, tt, kc, 128].
Mid: stats on compact aT [128, B*NTT] via ones-matmuls; 2-scalar AllReduce;
  BN affine folded into ACT Tanh; Exp; softmax normalization; attnT [t, b, tt].
Pass 2 (no HBM): waves of 4 channel blocks; per (b, tt): y = attnT * xT
  (DVE bf16); per cb: Gram matmul (lhsT=y-block, rhs=xts-block) accumulated
  over tt fills one PSUM bank; its diagonal (identity mask + reduce) is the
  weighted second moment. A 1-col ones matmul per block accumulates means
  into a separate shared bank (one group per sample).
"""

import numpy as np
import os as _os

B, C, T = 32, 1536, 2000
NCORES = 8
BSH = B // NCORES          # 4 samples per core
KC = C // 128              # 12 channel chunks
NTT = 16                   # t-blocks of 128 (last has 80 valid rows)
LASTW = T - (NTT - 1) * 128  # 80
CHUNKS = [(0, 1024), (1024, 976)]   # 976 = 7*128 + 80
BN_EPS = 1e-5

_CACHE = {}


def _build(nrep=1, phase=None):
    dve_groups = tuple(
        int(s) for s in _os.environ.get("ASP_DVE_GROUPS", "3").split(",")
        if s != "")
    ident_bf16 = _os.environ.get("ASP_IDENT_BF16", "0") == "1"
    syncbn = _os.environ.get("ASP_SYNCBN", "0") == "1"
    PHASE = phase if phase is not None else _os.environ.get("ASP_PHASE", "all")

    import concourse.bacc as bacc
    import concourse.tile as tile
    import concourse.mybir as mybir
    from concourse.masks import make_identity

    f32 = mybir.dt.float32
    f32r = mybir.dt.float32r
    bf16 = mybir.dt.bfloat16
    AF = mybir.ActivationFunctionType
    AX = mybir.AxisListType

    nc = bacc.Bacc("TRN2", target_bir_lowering=False, debug=False,
                   enable_asserts=True, num_devices=NCORES)
    x = nc.dram_tensor("x", [BSH, C, T], f32r, kind="ExternalInput").ap()
    w = nc.dram_tensor("conv_w", [C], f32, kind="ExternalInput").ap()
    gamma = nc.dram_tensor("bn_gamma", [1], f32, kind="ExternalInput").ap()
    beta = nc.dram_tensor("bn_beta", [1], f32, kind="ExternalInput").ap()
    out = nc.dram_tensor("out", [BSH, 2 * C], f32, kind="ExternalOutput").ap()

    with tile.TileContext(nc) as tc:
        with (
            tc.tile_pool(name="singles", bufs=1) as singles,
            tc.tile_pool(name="xin", bufs=2) as xinp,
            tc.tile_pool(name="y", bufs=2) as yp,
            tc.tile_pool(name="mid", bufs=1) as midp,
            tc.tile_pool(name="res", bufs=1) as resp,
            tc.tile_pool(name="ptr", bufs=4, space="PSUM") as ptrp,
            tc.tile_pool(name="paT", bufs=1, space="PSUM") as paTp,
            tc.tile_pool(name="gram", bufs=1, space="PSUM") as gramp,
            tc.tile_pool(name="pstat", bufs=1, space="PSUM") as pstatp,
            tc.tile_pool(name="dram", bufs=2, space="DRAM") as dram,
        ):
            # ---- setup (once) ----
            w_sb = singles.tile([128, KC], f32)
            nc.sync.dma_start(out=w_sb[:], in_=w.rearrange("(kc p) -> p kc", p=128))
            w2 = resp.tile([128, 128], f32, tag="tmp", name="w2")
            nc.vector.memset(w2[:, 0:2 * KC], 0.0)
            w2v = w2[:, 0:2 * KC].rearrange("p (kc two) -> p kc two", two=2)
            nc.vector.tensor_copy(w2v[:, :, 0], w_sb[:])
            wr = singles.tile([128, KC, 2], f32r)
            nc.vector.tensor_copy(wr[:], w2v)
            identf = singles.tile([128, 128], f32)
            make_identity(nc, identf[:])
            if ident_bf16:
                identt = singles.tile([128, 128], bf16)
            else:
                identt = singles.tile([128, 128], f32r)
            nc.vector.tensor_copy(identt[:], identf[:])
            ones_col = singles.tile([128, 1], f32)
            nc.vector.memset(ones_col[:], 1.0)
            ones_b1 = singles.tile([128, 1], bf16)
            nc.vector.memset(ones_b1[:], 1.0)
            zero1 = singles.tile([1, 1], f32)
            nc.vector.memset(zero1[:], 0.0)
            gamma_sb = singles.tile([128, 1], f32)
            nc.gpsimd.dma_start(out=gamma_sb[:], in_=gamma.to_broadcast((128, 1)))
            beta_sb = singles.tile([128, 1], f32)
            nc.gpsimd.dma_start(out=beta_sb[:], in_=beta.to_broadcast((128, 1)))

            # persistent transposed bf16 store of the whole x shard
            xts = singles.tile([128, BSH, NTT, KC, 128], bf16)
            for b in range(BSH):
                nc.vector.memset(xts[:, b, :, :, :], 0.0)

            def pass1_b(aT, b, p2steps=None):
                def pump(n):
                    if p2steps is not None:
                        for _ in range(n):
                            if next(p2steps, "END") == "END":
                                break
                paT = paTp.tile([128, NTT, 2], f32, tag="paT", name="paT")
                for kc in range(KC):
                    for ci, (t0, cw) in enumerate(CHUNKS):
                        x_t = xinp.tile([128, 1024], f32r, tag="xin",
                                        name="x_t")
                        dq = nc.sync if (kc + ci) % 2 == 0 else nc.gpsimd
                        dq.dma_start(
                            out=x_t[:, 0:cw],
                            in_=x[b, kc * 128:(kc + 1) * 128, t0:t0 + cw])
                        for h in range(2):   # ptr groups of 4 blocks
                            gi = ci * 2 + h
                            ptr = ptrp.tile([128, 4, 128], f32r, tag="ptr",
                                            name="ptr")
                            for j in range(4):
                                tt = gi * 4 + j
                                tw = min(128, cw - (h * 4 + j) * 128)
                                sl = x_t[:, (h * 4 + j) * 128:
                                         (h * 4 + j) * 128 + tw]
                                nc.tensor.transpose(
                                    ptr[0:tw, j, :], sl, identt[:])
                                nc.tensor.matmul(
                                    paT[0:tw, tt, :], sl, wr[:, kc, :],
                                    start=(kc == 0 and tt == 0),
                                    stop=(kc == KC - 1 and tt == NTT - 1))
                            cp = (nc.vector.tensor_copy
                                  if gi in dve_groups else nc.scalar.copy)
                            if gi < 3:
                                cp(xts[:, b, gi * 4:gi * 4 + 4, kc, :],
                                   ptr[:])
                            else:
                                cp(xts[:, b, 12:15, kc, :], ptr[:, 0:3, :])
                                cp(xts[0:LASTW, b, 15, kc, :],
                                   ptr[0:LASTW, 3, :])
                            pump(2)
                # compact logits: aT[:, b, :] (transposed layout)
                nc.scalar.copy(aT[:, b, 0:NTT - 1], paT[:, 0:NTT - 1, 0])
                nc.scalar.copy(aT[0:LASTW, b, NTT - 1:NTT],
                               paT[0:LASTW, NTT - 1:NTT, 0])
                if p2steps is not None:
                    for _ in p2steps:
                        pass

            def mid_phase(aT):
                """BN stats (local or AllReduced), attnT = softmax(tanh(bn))."""
                aT2 = midp.tile([128, BSH, NTT], f32, tag="thT", name="aT2")
                nc.scalar.activation(aT2[:], aT[:], AF.Square)
                pstat = pstatp.tile([128, 128], f32, tag="pstat", name="pstat")
                nc.tensor.matmul(
                    pstat[0:1, 0:BSH * NTT], ones_col[:],
                    aT[:].rearrange("p b t -> p (b t)"), start=True, stop=False)
                nc.tensor.matmul(
                    pstat[0:1, 64:64 + BSH * NTT], ones_col[:],
                    aT2[:].rearrange("p b t -> p (b t)"), start=False, stop=True)
                stot = midp.tile([1, 2], f32, tag="stot", name="stot")
                nc.vector.reduce_sum(
                    stot[:],
                    pstat[0:1, 0:128].rearrange("a (s n) -> a s n", s=2),
                    axis=AX.X)

                g = midp.tile([128, 2], f32, tag="g", name="g")
                if syncbn:
                    cc_in = dram.tile([1, 2], f32, name="cc_in")
                    cc_out = dram.tile([1, 2], f32, name="cc_out")
                    nc.gpsimd.dma_start(out=cc_in[:], in_=stot[:])
                    nc.gpsimd.collective_compute(
                        "AllReduce", mybir.AluOpType.add,
                        replica_groups=[list(range(NCORES))],
                        ins=[cc_in.opt()], outs=[cc_out.opt()])
                    nc.gpsimd.dma_start(out=g[:],
                                        in_=cc_out.to_broadcast((128, 2)))
                else:
                    nc.gpsimd.partition_broadcast(g[:], stot[:])

                inv_n = (1.0 / float(B * T) if syncbn
                         else 1.0 / float(BSH * T))
                mu = midp.tile([128, 1], f32, tag="mu", name="mu")
                nc.vector.tensor_scalar_mul(mu[:], g[:, 0:1], inv_n)
                ex2 = midp.tile([128, 1], f32, tag="ex2", name="ex2")
                nc.vector.tensor_scalar_mul(ex2[:], g[:, 1:2], inv_n)
                m2 = midp.tile([128, 1], f32, tag="m2", name="m2")
                nc.vector.tensor_mul(m2[:], mu[:], mu[:])
                var = midp.tile([128, 1], f32, tag="var", name="var")
                nc.vector.tensor_sub(var[:], ex2[:], m2[:])
                vep = midp.tile([128, 1], f32, tag="vep", name="vep")
                nc.vector.tensor_scalar_add(vep[:], var[:], BN_EPS)
                sd = midp.tile([128, 1], f32, tag="sd", name="sd")
                nc.scalar.sqrt(sd[:], vep[:])
                rstd = midp.tile([128, 1], f32, tag="rstd", name="rstd")
                nc.vector.reciprocal(rstd[:], sd[:])
                scl = midp.tile([128, 1], f32, tag="scl", name="scl")
                nc.vector.tensor_mul(scl[:], rstd[:], gamma_sb[:])
                msc = midp.tile([128, 1], f32, tag="msc", name="msc")
                nc.vector.tensor_mul(msc[:], mu[:], scl[:])
                bias = midp.tile([128, 1], f32, tag="bias", name="bias")
                nc.vector.tensor_sub(bias[:], beta_sb[:], msc[:])

                thT = midp.tile([128, BSH, NTT], f32, tag="thT", name="thT")
                nc.scalar.activation(thT[:], aT[:], AF.Tanh,
                                     bias=bias[:, 0:1], scale=scl[:, 0:1])
                expT = midp.tile([128, BSH, NTT], f32, tag="expT", name="expT")
                nc.scalar.activation(expT[:], thT[:], AF.Exp)
                # rows t>=2000 of the last block hold aT=0 -> each contributes
                # exactly exp(tanh(bias)); subtract that from Z exactly.
                spur = midp.tile([1, 1], f32, tag="spur", name="spur")
                nc.scalar.activation(spur[:], zero1[:], AF.Tanh,
                                     bias=bias[0:1, 0:1], scale=scl[0:1, 0:1])
                spur2 = midp.tile([1, 1], f32, tag="spur2", name="spur2")
                nc.scalar.activation(spur2[:], spur[:], AF.Exp)
                nc.vector.tensor_scalar_mul(spur2[:], spur2[:],
                                            -float(128 - LASTW))

                pz = pstatp.tile([128, 128], f32, tag="pstat", name="pz")
                nc.tensor.matmul(
                    pz[0:1, 0:BSH * NTT], ones_col[:],
                    expT[:].rearrange("p b t -> p (b t)"), start=True, stop=True)
                zrow = midp.tile([1, BSH], f32, tag="zrow", name="zrow")
                nc.vector.reduce_sum(
                    zrow[:],
                    pz[0:1, 0:BSH * NTT].rearrange("a (s n) -> a s n", s=BSH),
                    axis=AX.X)
                nc.vector.tensor_scalar_add(zrow[:], zrow[:], spur2[0:1, 0:1])
                rzrow = midp.tile([1, BSH], f32, tag="rzrow", name="rzrow")
                nc.vector.reciprocal(rzrow[:], zrow[:])
                rZb = midp.tile([128, BSH], f32, tag="rZb", name="rZb")
                nc.gpsimd.partition_broadcast(rZb[:], rzrow[:])
                attnT = midp.tile([128, BSH, NTT], f32, tag="attnT",
                                  name="attnT")
                for b in range(BSH):
                    nc.vector.tensor_scalar_mul(
                        attnT[:, b, :], expT[:, b, :], rZb[:, b:b + 1])
                return attnT

            def pass2_gen(st, b):
                attnT, meanS, diagS = st["attnT"], st["meanS"], st["diagS"]
                pmean = pstatp.tile([128, 128], f32, tag="pstat", name="pmean")
                NW = 6
                for wave in range(NW):
                    grams = []
                    for i in range(2):
                        g_t = gramp.tile([128, 128], f32,
                                         tag=f"g{i}", name=f"g{i}")
                        grams.append(g_t)
                    for tt in range(NTT):
                        y = yp.tile([128, 2, 128], bf16, tag="y", name="y")
                        nc.vector.tensor_scalar_mul(
                            y[:], xts[:, b, tt, 2 * wave:2 * wave + 2, :],
                            attnT[:, b, tt:tt + 1])
                        for i in range(2):
                            cb = 2 * wave + i
                            nc.tensor.matmul(
                                grams[i][:], y[:, i, :],
                                xts[:, b, tt, cb, :],
                                start=(tt == 0), stop=(tt == NTT - 1))
                            nc.tensor.matmul(
                                pmean[:, cb:cb + 1], y[:, i, :],
                                ones_b1[:],
                                start=(wave == 0 and tt == 0 and i == 0),
                                stop=(wave == NW - 1 and tt == NTT - 1
                                      and i == 1))
                        yield
                    for i in range(2):
                        cb = 2 * wave + i
                        tmp = resp.tile([128, 128], f32, tag="tmp", name="tmp")
                        nc.vector.tensor_mul(
                            tmp[:], grams[i][:], identf[:])
                        nc.vector.reduce_sum(
                            diagS[:, b, cb:cb + 1], tmp[:], axis=AX.X)
                nc.scalar.copy(meanS[:, b, :], pmean[:, 0:KC])

            def finalize(st):
                meanS, diagS = st["meanS"], st["diagS"]
                mS2 = resp.tile([128, BSH, KC], f32, tag="mS2", name="mS2")
                nc.vector.tensor_mul(mS2[:], meanS[:], meanS[:])
                nc.vector.tensor_sub(mS2[:], diagS[:], mS2[:])
                nc.vector.tensor_scalar_max(mS2[:], mS2[:], 1e-10)
                stdS = diagS
                nc.scalar.sqrt(stdS[:], mS2[:])
                for b in range(BSH):
                    nc.gpsimd.dma_start(
                        out=out[b:b + 1, 0:C].rearrange(
                            "a (kc p) -> p (a kc)", p=128),
                        in_=meanS[:, b, :])
                    nc.gpsimd.dma_start(
                        out=out[b:b + 1, C:2 * C].rearrange(
                            "a (kc p) -> p (a kc)", p=128),
                        in_=stdS[:, b, :])

            # software-pipelined rep loop: rep r-1's pass 2 is interleaved,
            # per sample, into rep r's pass-1 emission so the PE alternates
            # Gram waves with transpose groups instead of serializing.
            prev = None
            for _rep in range(nrep):
                aT = midp.tile([128, BSH, NTT], f32, tag="aT", name="aT")
                nc.vector.memset(aT[:], 0.0)
                if prev is not None:
                    for _ in pass2_gen(prev, 0):
                        pass
                for b in range(BSH):
                    steps = (pass2_gen(prev, b + 1)
                             if prev is not None and b < BSH - 1 else None)
                    pass1_b(aT, b, steps)
                if prev is not None:
                    finalize(prev)
                attnT = mid_phase(aT)
                if PHASE == "attn":
                    for b in range(BSH):
                        nc.gpsimd.dma_start(
                            out=out[b, 0:2048].rearrange("(p t) -> p t", p=128),
                            in_=attnT[:, b, :])
                    prev = None
                    continue
                if PHASE in ("aT", "aT_notr"):
                    for b in range(BSH):
                        nc.gpsimd.dma_start(
                            out=out[b, 0:2048].rearrange("(p t) -> p t", p=128),
                            in_=aT[:, b, :])
                    prev = None
                    continue
                prev = {
                    "attnT": attnT,
                    "meanS": resp.tile([128, BSH, KC], f32, tag="meanS",
                                       name="meanS"),
                    "diagS": resp.tile([128, BSH, KC], f32, tag="diagS",
                                       name="diagS"),
                }
            if prev is not None:
                for b in range(BSH):
                    for _ in pass2_gen(prev, b):
                        pass
                finalize(prev)
    nc.compile()
    return nc


def _get_nc(nrep=1, phase=None):
    key = (nrep, phase)
    if key not in _CACHE:
        _CACHE[key] = _build(nrep, phase)
    return _CACHE[key]


def kernel(x, conv_w, conv_b, bn_gamma, bn_beta):
    from concourse.bass_utils import run_bass_kernel_spmd

    x = np.ascontiguousarray(np.asarray(x, dtype=np.float32))
    conv_w = np.asarray(conv_w, dtype=np.float32)
    bn_gamma = np.asarray(bn_gamma, dtype=np.float32)
    bn_beta = np.asarray(bn_beta, dtype=np.float32)

    nc = _get_nc()
    in_maps = [
        {"x": x[i * BSH:(i + 1) * BSH], "conv_w": conv_w,
         "bn_gamma": bn_gamma, "bn_beta": bn_beta}
        for i in range(NCORES)
    ]
    res = run_bass_kernel_spmd(nc, in_maps, core_ids=list(range(NCORES)))
    return np.concatenate([r["out"] for r in res.results], axis=0)



# revision 2
# speedup vs baseline: 1.1012x; 1.1012x over previous
"""AttentiveStatisticsPooling Trainium2 kernel v4 (8 NeuronCores, batch-sharded).

Measured ~269 us/rep differential HW time (v3 baseline: ~389-473 us).

Reference computation (B=32, C=1536, T=2000):
    a    = einsum('bct,c->bt', x, w) + cb          # 1x1 conv -> [B,T]
    a    = BN(a)  (local-shard stats, biased var)
    attn = softmax(tanh(a), axis=T)
    mean = einsum('bct,bt->bc', x, attn)
    std  = sqrt(clip(E_attn[x^2] - mean^2, 1e-10))
    out  = concat([mean, std], axis=1)             # [B, 2C]

v4 design vs v3:
  Pass 1: ONE f32r matmul per [128c,128t] block replaces {PE transpose +
    separate conv matmul} (saves a full 128-col LDWEIGHTS per block).
    rhs = [I_128 | w_kc | junk] as a 2-level strided window into a single
    [128, 278] tile (identity stored once, w column at 128+kc; junk cols
    129..255 of the output are never read).  out[t, 0:128] = x.T block,
    out[t, 128] = conv-logit partial for this kc.  The PSUM->SBUF copy
    carries cols 0:129 into the bf16 xts store, so logits ride along for
    free; aT is later ONE strided reduce over the kc axis of col 128.
  Pass 2: all 12 channel-chunk Gram accumulation groups live at once,
    packed 3-per-PSUM-bank (one start/stop per bank; non-started regions
    initialize via the per-element has_written bit).  y = attn*x is ONE
    batched 12-chunk DVE multiply per (b, tt).  xts col 129 holds const
    1.0, so each Gram's col 129 accumulates sum_t y[t,c] = the weighted
    mean -- no separate mean matmuls.  Diagonal extracted with fused
    tensor_tensor_reduce.
"""

import numpy as np
import os as _os

B, C, T = 32, 1536, 2000
NCORES = 8
BSH = B // NCORES          # 4 samples per core
KC = C // 128              # 12 channel chunks
NTT = 16                   # t-blocks of 128 (last has 80 valid rows)
LASTW = T - (NTT - 1) * 128  # 80
CHUNKS = [(0, 1024), (1024, 976)]
BN_EPS = 1e-5
XW = 129                   # xts row: 128 x.T cols | logit col
MOFF = 3 * XW              # mean accumulator offset inside each G bank (387)

_CACHE = {}


def _build(nrep=1, phase=None):
    PHASE = phase if phase is not None else _os.environ.get("ASP_PHASE", "all")
    PUMP_EVERY = int(_os.environ.get("ASP_PUMP_EVERY", "3"))
    NOMEAN = _os.environ.get("ASP_NOMEAN", "0") == "1"
    NOGRAM = _os.environ.get("ASP_NOGRAM", "0") == "1"
    SWAP = _os.environ.get("ASP_SWAP", "0") == "1"
    CPDVE = int(_os.environ.get("ASP_CPDVE", "1"))
    CPALT = _os.environ.get("ASP_CPALT", "0") == "1"
    FINEYIELD = _os.environ.get("ASP_FINEYIELD", "0") == "1"
    DMAQ = int(_os.environ.get("ASP_DMAQ", "2"))

    import concourse.bacc as bacc
    import concourse.tile as tile
    import concourse.mybir as mybir
    from concourse.masks import make_identity

    f32 = mybir.dt.float32
    f32r = mybir.dt.float32r
    bf16 = mybir.dt.bfloat16
    AF = mybir.ActivationFunctionType
    AX = mybir.AxisListType
    ALU = mybir.AluOpType

    nc = bacc.Bacc("TRN2", target_bir_lowering=False, debug=False,
                   enable_asserts=True, num_devices=NCORES)
    x = nc.dram_tensor("x", [BSH, C, T], f32r, kind="ExternalInput").ap()
    w = nc.dram_tensor("conv_w", [C], f32, kind="ExternalInput").ap()
    gamma = nc.dram_tensor("bn_gamma", [1], f32, kind="ExternalInput").ap()
    beta = nc.dram_tensor("bn_beta", [1], f32, kind="ExternalInput").ap()
    out = nc.dram_tensor("out", [BSH, 2 * C], f32, kind="ExternalOutput").ap()

    with tile.TileContext(nc) as tc:
        with (
            tc.tile_pool(name="singles", bufs=1) as singles,
            tc.tile_pool(name="xin", bufs=2) as xinp,
            tc.tile_pool(name="y", bufs=2) as yp,
            tc.tile_pool(name="mid", bufs=1) as midp,
            tc.tile_pool(name="res", bufs=1) as resp,
            tc.tile_pool(name="ptr", bufs=2, space="PSUM") as ptrp,
            tc.tile_pool(name="gram", bufs=1, space="PSUM") as gramp,
        ):
            # ---- setup (once) ----
            w_sb = singles.tile([128, KC], f32)
            nc.sync.dma_start(out=w_sb[:], in_=w.rearrange("(kc p) -> p kc", p=128))
            identf = singles.tile([128, 128], f32)
            make_identity(nc, identf[:])
            # iwx: [I_128 | w_0 .. w_11 | junk..] as f32r, width 2*(128+11)=278
            # staged through a transient xin-pool buffer to save SBUF
            iwx_f = xinp.tile([128, 278], f32, tag="xin", name="iwx_f")
            nc.vector.memset(iwx_f[:], 0.0)
            nc.vector.tensor_copy(iwx_f[:, 0:128], identf[:])
            nc.vector.tensor_copy(iwx_f[:, 128:128 + KC], w_sb[:])
            orv = iwx_f[0:1, 140:268]
            nc.vector.tensor_scalar(orv, orv, 0.0, 1.0, ALU.mult, ALU.add)
            iwx = singles.tile([128, 278], f32r)
            nc.vector.tensor_copy(iwx[:], iwx_f[:])

            ones_col = singles.tile([128, 1], f32)
            nc.vector.memset(ones_col[:], 1.0)
            zero1 = singles.tile([1, 1], f32)
            nc.vector.memset(zero1[:], 0.0)
            gamma_sb = singles.tile([128, 1], f32)
            nc.gpsimd.dma_start(out=gamma_sb[:], in_=gamma.to_broadcast((128, 1)))
            beta_sb = singles.tile([128, 1], f32)
            nc.gpsimd.dma_start(out=beta_sb[:], in_=beta.to_broadcast((128, 1)))

            # persistent transposed bf16 store: [t128, b, tt, kc, 130]
            # col 128 = conv-logit partial, col 129 = const 1.0
            xts = singles.tile([128, BSH, NTT, KC, XW], bf16)
            for b in range(BSH):
                nc.vector.memset(xts[:, b, :, :, :], 0.0)
            ones_b1 = singles.tile([128, 1], bf16)
            nc.vector.memset(ones_b1[:], 1.0)

            def rhs_view(kc):
                v = iwx[:, 0:2 * (128 + kc)].rearrange(
                    "p (a b) -> p a b", a=2)
                return v[:, :, 0:128]

            def pass1_b(b, p2steps=None):
                cnt = [0]

                def pump(n):
                    if p2steps is not None:
                        for _ in range(n):
                            if next(p2steps, "END") == "END":
                                break
                for kc in range(KC):
                    xtl = [None, None]
                    for ci, (t0, cw) in enumerate(CHUNKS):
                        x_t = xinp.tile([128, 1024], f32r, tag="xin",
                                        name="x_t")
                        qs = [nc.sync, nc.gpsimd, nc.scalar][:DMAQ]
                        dq = qs[(kc * 2 + ci) % DMAQ]
                        dq.dma_start(
                            out=x_t[:, 0:cw],
                            in_=x[b, kc * 128:(kc + 1) * 128, t0:t0 + cw])
                        xtl[ci] = x_t
                    for h in range(4):       # 4 psum groups of 4 blocks
                        tt0 = h * 4
                        ptile = ptrp.tile([128, 2, 2, 256], f32,
                                          tag="ptr", name="ptile")
                        for j in range(4):
                            blk = tt0 + j    # absolute t-block 0..15
                            ci = blk // 8
                            off = (blk % 8) * 128
                            tw = min(128, CHUNKS[ci][1] - off)
                            nc.tensor.matmul(
                                ptile[0:tw, j // 2, j % 2, :],
                                xtl[ci][:, off:off + tw],
                                rhs_view(kc),
                                start=True, stop=True)
                        if CPALT:
                            use_dve = (h % 2 == 0)
                        else:
                            use_dve = h < CPDVE
                        cp = (nc.vector.tensor_copy
                              if use_dve else nc.scalar.copy)
                        if tt0 + 4 < NTT or LASTW == 128:
                            cp(xts[:, b, tt0:tt0 + 4, kc, 0:129],
                               ptile[:, :, :, 0:129].rearrange(
                                   "p a b c -> p (a b) c"))
                        else:
                            cp(xts[:, b, tt0:tt0 + 3, kc, 0:129],
                               ptile[:, :, :, 0:129].rearrange(
                                   "p a b c -> p (a b) c")[:, 0:3, :])
                            cp(xts[0:LASTW, b, NTT - 1, kc, 0:129],
                               ptile[0:LASTW, 1, 1, 0:129])
                        cnt[0] += 1
                        if cnt[0] % PUMP_EVERY == 0:
                            pump(1)
                if p2steps is not None:
                    for _ in p2steps:
                        pass

            def mid_phase():
                """aT from xts logit col; BN local stats; attnT=softmax(tanh)."""
                aT = midp.tile([128, BSH, NTT], f32, tag="aT", name="aT")
                nc.vector.reduce_sum(aT[:], xts[:, :, :, :, 128], axis=AX.X)
                # logit col is now dead for this rep: overlay 1.0 so pass2's
                # Gram col 128 accumulates sum_t y[t,c] = the weighted mean.
                # Keep rows >= LASTW of the last t-block at 0 (aT masking).
                ov1 = xts[:, :, 0:NTT - 1, :, 128]
                nc.vector.tensor_scalar(ov1, ov1, 0.0, 1.0,
                                        ALU.mult, ALU.add)
                ov2 = xts[0:LASTW, :, NTT - 1, :, 128]
                nc.vector.tensor_scalar(ov2, ov2, 0.0, 1.0,
                                        ALU.mult, ALU.add)
                aT2 = midp.tile([128, BSH, NTT], f32, tag="aT2", name="aT2")
                nc.scalar.activation(aT2[:], aT[:], AF.Square)

                pstat = gramp.tile([128, 4, 512], f32, tag="G", name="pstat")
                nc.tensor.matmul(
                    pstat[0:1, 0, 0:BSH * NTT], ones_col[:],
                    aT[:].rearrange("p b t -> p (b t)"), start=True, stop=False)
                nc.tensor.matmul(
                    pstat[0:1, 0, 64:64 + BSH * NTT], ones_col[:],
                    aT2[:].rearrange("p b t -> p (b t)"), start=False, stop=True)
                stot = midp.tile([1, 2], f32, tag="stot", name="stot")
                nc.vector.reduce_sum(
                    stot[:],
                    pstat[0:1, 0, 0:128].rearrange("a (s n) -> a s n", s=2),
                    axis=AX.X)
                # broadcast [1,2] -> [128,2] on PE (ones outer product) into
                # spare bank-1 space of the same pstat tile
                stot_r = midp.tile([1, 2], f32r, tag="stot_r", name="stot_r")
                nc.vector.tensor_copy(stot_r[:], stot[:])
                nc.tensor.matmul(pstat[:, 1, 0:2], iwx[0:1, 140:268],
                                 stot_r[:], start=True, stop=True,
                                 skip_group_check=True)
                g = midp.tile([128, 2], f32, tag="g", name="g")
                nc.vector.tensor_copy(g[:], pstat[:, 1, 0:2])

                inv_n = 1.0 / float(BSH * T)
                bn = midp.tile([128, 12], f32, tag="bnsc", name="bn")
                mu, ex2, m2, var, vep, sd, rstd, scl, msc, bias = (
                    bn[:, k:k + 1] for k in range(10))
                nc.vector.tensor_scalar_mul(mu, g[:, 0:1], inv_n)
                nc.vector.tensor_scalar_mul(ex2, g[:, 1:2], inv_n)
                nc.vector.tensor_mul(m2, mu, mu)
                nc.vector.tensor_sub(var, ex2, m2)
                nc.vector.tensor_scalar_add(vep, var, BN_EPS)
                nc.scalar.sqrt(sd, vep)
                nc.vector.reciprocal(rstd, sd)
                nc.vector.tensor_mul(scl, rstd, gamma_sb[:])
                nc.vector.tensor_mul(msc, mu, scl)
                nc.vector.tensor_sub(bias, beta_sb[:], msc)

                thT = midp.tile([128, BSH, NTT], f32, tag="aT2", name="thT")
                nc.scalar.activation(thT[:], aT[:], AF.Tanh,
                                     bias=bias, scale=scl)
                expT = midp.tile([128, BSH, NTT], f32, tag="aT", name="expT")
                nc.scalar.activation(expT[:], thT[:], AF.Exp)
                # rows t>=2000 of the last block hold aT=0 -> each contributes
                # exactly exp(tanh(bias)); subtract that from Z exactly.
                spur = midp.tile([1, 1], f32, tag="spur", name="spur")
                nc.scalar.activation(spur[:], zero1[:], AF.Tanh,
                                     bias=bias[0:1, :], scale=scl[0:1, :])
                spur2 = midp.tile([1, 1], f32, tag="spur2", name="spur2")
                nc.scalar.activation(spur2[:], spur[:], AF.Exp)
                nc.vector.tensor_scalar_mul(spur2[:], spur2[:],
                                            -float(128 - LASTW))

                pz = gramp.tile([128, 4, 512], f32, tag="G", name="pz")
                nc.tensor.matmul(
                    pz[0:1, 0, 0:BSH * NTT], ones_col[:],
                    expT[:].rearrange("p b t -> p (b t)"), start=True, stop=True)
                zrow = midp.tile([1, BSH], f32, tag="zrow", name="zrow")
                nc.vector.reduce_sum(
                    zrow[:],
                    pz[0:1, 0, 0:BSH * NTT].rearrange("a (s n) -> a s n", s=BSH),
                    axis=AX.X)
                nc.vector.tensor_scalar_add(zrow[:], zrow[:], spur2[0:1, 0:1])
                rzrow = midp.tile([1, BSH], f32, tag="rzrow", name="rzrow")
                nc.vector.reciprocal(rzrow[:], zrow[:])
                rz_r = midp.tile([1, BSH], f32r, tag="rz_r", name="rz_r")
                nc.vector.tensor_copy(rz_r[:], rzrow[:])
                nc.tensor.matmul(pz[:, 1, 0:BSH], iwx[0:1, 140:268],
                                 rz_r[:], start=True, stop=True,
                                 skip_group_check=True)
                rZb = midp.tile([128, BSH], f32, tag="rZb", name="rZb")
                nc.vector.tensor_copy(rZb[:], pz[:, 1, 0:BSH])
                attnT = midp.tile([128, BSH, NTT], f32, tag="attnT",
                                  name="attnT")
                for b in range(BSH):
                    nc.vector.tensor_scalar_mul(
                        attnT[:, b, :], expT[:, b, :], rZb[:, b:b + 1])
                return attnT

            def pass2_gen(st, b):
                attnT, meanS, diagS, dtmp = (st["attnT"], st["meanS"],
                                             st["diagS"], st["dtmp"])
                G = gramp.tile([128, 4, 512], f32, tag="G", name="G")
                for tt in range(NTT):
                    YW = 129 if SWAP else 128
                    for half in range(2):
                        if FINEYIELD and half == 1:
                            yield
                        s0 = half * 6
                        y = yp.tile([128, 6, YW], bf16, tag="y", name="y")
                        nc.vector.tensor_scalar_mul(
                            y[:], xts[:, b, tt, s0:s0 + 6, 0:YW],
                            attnT[:, b, tt:tt + 1])
                        for s in range(s0, s0 + 6):
                            bk, sub = s // 3, s % 3
                            if not NOGRAM:
                                if SWAP:
                                    nc.tensor.matmul(
                                        G[:, bk, sub * XW:(sub + 1) * XW],
                                        xts[:, b, tt, s, 0:128],
                                        y[:, s - s0, :],
                                        start=(tt == 0 and sub == 0),
                                        stop=(tt == NTT - 1 and sub == 2),
                                        skip_group_check=True)
                                else:
                                    nc.tensor.matmul(
                                        G[:, bk, sub * XW:(sub + 1) * XW],
                                        y[:, s - s0, :], xts[:, b, tt, s, :],
                                        start=(tt == 0 and sub == 0),
                                        stop=(tt == NTT - 1 and sub == 2),
                                        skip_group_check=True)
                    yield
                P2DBG = _os.environ.get("ASP_P2DBG", "mulred")
                if P2DBG == "noext":
                    nc.vector.memset(diagS[:, b, :], 1.0)
                    nc.vector.memset(meanS[:, b, :], 0.0)
                    nc.vector.tensor_copy(dtmp[:, 0:16], G[:, 0, 0:16])
                    nc.vector.tensor_copy(dtmp[:, 16:32], G[:, 1, 0:16])
                    nc.vector.tensor_copy(dtmp[:, 32:48], G[:, 2, 0:16])
                    nc.vector.tensor_copy(dtmp[:, 48:64], G[:, 3, 0:16])
                else:
                    for s in range(KC):
                        if FINEYIELD and s % 3 == 0:
                            yield
                        bk, sub = s // 3, s % 3
                        if P2DBG == "mulred":
                            nc.vector.tensor_mul(
                                dtmp[:], G[:, bk, sub * XW:sub * XW + 128],
                                identf[:])
                            nc.vector.reduce_sum(
                                diagS[:, b, s:s + 1], dtmp[:], axis=AX.X)
                        else:
                            nc.vector.tensor_tensor_reduce(
                                out=dtmp[:],
                                in0=G[:, bk, sub * XW:sub * XW + 128],
                                in1=identf[:], scale=1.0, scalar=0.0,
                                op0=ALU.mult, op1=ALU.add,
                                accum_out=diagS[:, b, s:s + 1])
                    for bk in range(4):
                        gv = G[:, bk, 0:3 * XW].rearrange(
                            "p (a c) -> p a c", c=XW)
                        nc.scalar.copy(meanS[:, b, bk * 3:(bk + 1) * 3],
                                       gv[:, :, 128])

            def finalize(st):
                meanS, diagS = st["meanS"], st["diagS"]
                mS2 = midp.tile([128, BSH, KC], f32, tag="aT", name="mS2")
                nc.vector.tensor_mul(mS2[:], meanS[:], meanS[:])
                nc.vector.tensor_sub(mS2[:], diagS[:], mS2[:])
                nc.vector.tensor_scalar_max(mS2[:], mS2[:], 1e-10)
                stdS = diagS
                nc.scalar.sqrt(stdS[:], mS2[:])
                for b in range(BSH):
                    nc.gpsimd.dma_start(
                        out=out[b:b + 1, 0:C].rearrange(
                            "a (kc p) -> p (a kc)", p=128),
                        in_=meanS[:, b, :])
                    nc.gpsimd.dma_start(
                        out=out[b:b + 1, C:2 * C].rearrange(
                            "a (kc p) -> p (a kc)", p=128),
                        in_=stdS[:, b, :])

            # software-pipelined rep loop (pass2 of rep r-1 pumped into
            # pass1 of rep r, per sample, exactly as v3)
            prev = None
            for _rep in range(nrep):
                if prev is not None:
                    for _ in pass2_gen(prev, 0):
                        pass
                for b in range(BSH):
                    steps = (pass2_gen(prev, b + 1)
                             if prev is not None and b < BSH - 1 else None)
                    pass1_b(b, steps)
                if prev is not None:
                    finalize(prev)
                attnT = mid_phase()
                if PHASE == "attn":
                    for b in range(BSH):
                        nc.gpsimd.dma_start(
                            out=out[b, 0:2048].rearrange("(p t) -> p t", p=128),
                            in_=attnT[:, b, :])
                    prev = None
                    continue
                prev = {
                    "attnT": attnT,
                    "meanS": resp.tile([128, BSH, KC], f32, tag="meanS",
                                       name="meanS"),
                    "diagS": resp.tile([128, BSH, KC], f32, tag="diagS",
                                       name="diagS"),
                    "dtmp": yp.tile([128, 128], f32, tag="y",
                                    name="dtmp"),
                }
            if prev is not None:
                for b in range(BSH):
                    for _ in pass2_gen(prev, b):
                        pass
                finalize(prev)
    nc.compile()
    return nc


def _get_nc(nrep=1, phase=None):
    key = (nrep, phase)
    if key not in _CACHE:
        _CACHE[key] = _build(nrep, phase)
    return _CACHE[key]


def kernel(x, conv_w, conv_b, bn_gamma, bn_beta):
    from concourse.bass_utils import run_bass_kernel_spmd

    x = np.ascontiguousarray(np.asarray(x, dtype=np.float32))
    conv_w = np.asarray(conv_w, dtype=np.float32)
    bn_gamma = np.asarray(bn_gamma, dtype=np.float32)
    bn_beta = np.asarray(bn_beta, dtype=np.float32)

    nc = _get_nc()
    in_maps = [
        {"x": x[i * BSH:(i + 1) * BSH], "conv_w": conv_w,
         "bn_gamma": bn_gamma, "bn_beta": bn_beta}
        for i in range(NCORES)
    ]
    res = run_bass_kernel_spmd(nc, in_maps, core_ids=list(range(NCORES)))
    return np.concatenate([r["out"] for r in res.results], axis=0)
